# revision 1
# baseline (speedup 1.0000x reference)
"""PointNet sampler (ball query + neighbor MLP + max-pool + per-center linear)
for Trainium2, sharded over 8 NeuronCores.

Full-input contract: kernel(**inputs) takes the complete arrays and returns the
complete (B, M, C_OUT) output. Internally the (batch, center) space is sharded
as core c -> batch c//2, centers half c%2 (512 centers per core).

Algorithm (per core):
  ball_query selects the first K=32 in-radius indices per center; for the
  spec's distance distribution these always lie in a PFX=256-column prefix of
  the distance rows, so the device scans only that prefix. Per-row valid
  counts within the prefix are returned to the host; any row whose count < K
  (never, for spec-conformant inputs) is recomputed exactly on host.

  The neighbor MLP is folded:  f[m,k,:] = H[n_k] - Cm'[m]  with
    H[n]  = [pos[n], feat[n]] @ W_op          (per point, PFX x 64)
    Cm'[m] = c_m @ W_op[:3] - b_op            (per center)
  so pooled = max_k H[n_k] - Cm'.

  The K-row max-gather runs on the TensorEngine: T = valid * cumsum(valid)
  marks slot j's point with value j (tensor_tensor_scan); slot indicator
  onehot_j[n, m] = (T^T[n, m] == j) streams as the matmul moving operand
  against the stationary H chunk, so PSUM receives H[n_j(m), :] per slot,
  which is max-accumulated - no DMA descriptors, no index extraction.
  Output = relu(pooled @ W_agg + b_agg) with the bias folded as an extra
  contraction row.
"""

import numpy as np

B, N, M = 4, 16384, 1024
D, C, C_OP, C_OUT, K = 3, 64, 64, 128, 32
R2 = 0.25
PFX = 256          # distance-prefix columns scanned on device
MC = M // 2        # centers per core (512)
NT = MC // 128     # 128-center tiles per core (4)
NXT = PFX // 128   # point chunks of the H table (2)
NCORES = 8
JG = 8             # slot groups of 4 (JG*4 == K)

_PROG = None


def _build_program(reps=0):
    import concourse.bacc as bacc
    import concourse.bass as bass
    import concourse.mybir as mybir
    import concourse.tile as tile
    from concourse.masks import make_identity

    f32 = mybir.dt.float32
    nc = bacc.Bacc(
        "TRN2", target_bir_lowering=False, debug=False, enable_asserts=False,
        num_devices=NCORES,
    )

    dist = nc.dram_tensor("dist", [MC, PFX], f32, kind="ExternalInput")
    xpfx = nc.dram_tensor("xpfx", [PFX, D + C], f32, kind="ExternalInput")
    cen = nc.dram_tensor("cen", [MC, D], f32, kind="ExternalInput")
    wop = nc.dram_tensor("wop", [D + C, C_OP], f32, kind="ExternalInput")
    w1b = nc.dram_tensor("w1b", [D + 1, C_OP], f32, kind="ExternalInput")
    waggb = nc.dram_tensor("waggb", [C_OP + 1, C_OUT], f32, kind="ExternalInput")
    out = nc.dram_tensor("out", [MC, C_OUT], f32, kind="ExternalOutput")
    cnt = nc.dram_tensor("cnt", [128, NT], f32, kind="ExternalOutput")

    with tile.TileContext(nc) as tc:
        with (
            tc.tile_pool(name="const", bufs=1) as const,
            tc.tile_pool(name="sb", bufs=2) as sb,
            tc.tile_pool(name="ohp", bufs=4) as ohp,
            tc.tile_pool(name="ps_t", bufs=1, space="PSUM") as ps_t,
            tc.tile_pool(name="ps_oh", bufs=5, space="PSUM") as ps_oh,
            tc.tile_pool(name="ps_o", bufs=1, space="PSUM") as ps_o,
        ):
            ident = const.tile([128, 128], f32)
            make_identity(nc, ident[:])

            zeros = const.tile([128, PFX], f32)
            nc.vector.memset(zeros[:], 0.0)

            # cj: slot-match constants, value 1 + f//128 at free position f
            cj = const.tile([128, 4 * JG * 128], f32)
            for s0 in range(4 * JG):
                nc.vector.memset(cj[:, s0 * 128:(s0 + 1) * 128], float(s0 + 1))

            wop_sb = const.tile([D + C, C_OP], f32)
            nc.sync.dma_start(wop_sb[:], wop[:])
            w1b_sb = const.tile([D + 1, C_OP], f32)
            nc.sync.dma_start(w1b_sb[:], w1b[:])
            waggb_sb = const.tile([C_OP + 1, C_OUT], f32)
            nc.sync.dma_start(waggb_sb[:], waggb[:])

            import contextlib as _ctx
            loop_ctx = tc.For_i(0, reps, 1) if reps else _ctx.nullcontext()
            with loop_ctx:
                # ---- H chunks: H[n] = [pos, feat] @ W_op  (SBUF resident) ----
                hc = []
                for xt in range(NXT):
                    x_sb = sb.tile([128, D + C], f32, tag="x")
                    nc.sync.dma_start(x_sb[:], xpfx[xt * 128:(xt + 1) * 128, :])
                    xT_ps = ps_t.tile([D + C, 128], f32, tag="tA")
                    nc.tensor.transpose(out=xT_ps[:], in_=x_sb[:], identity=ident[:])
                    xT_sb = sb.tile([D + C, 128], f32, tag="xT_sb")
                    nc.scalar.copy(xT_sb[:], xT_ps[:])
                    h_ps = ps_t.tile([128, C_OP], f32, tag="tB")
                    nc.tensor.matmul(out=h_ps[:], lhsT=xT_sb[:], rhs=wop_sb[:],
                                     start=True, stop=True)
                    h_sb = sb.tile([128, C_OP], f32, tag=f"hc{xt}")
                    nc.scalar.copy(h_sb[:], h_ps[:])
                    hc.append(h_sb)

                cnt_sb = sb.tile([128, NT], f32, tag="cnt")

                # ---- per 128-center tile ----
                for t in range(NT):
                    r0, r1 = t * 128, (t + 1) * 128

                    # Cm'^T = ([cx,cy,cz,-1] @ [W1; b_op])^T  -> (64, 128) PSUM
                    cen_sb = sb.tile([128, D + 1], f32, tag="cen")
                    nc.vector.memset(cen_sb[:, D:D + 1], -1.0)
                    nc.sync.dma_start(cen_sb[:, 0:D], cen[r0:r1, :])
                    cenT_ps = ps_t.tile([D + 1, 128], f32, tag="tA")
                    nc.tensor.transpose(out=cenT_ps[:], in_=cen_sb[:],
                                        identity=ident[:])
                    cenT_sb = sb.tile([D + 1, 128], f32, tag="cenT_sb")
                    nc.scalar.copy(cenT_sb[:], cenT_ps[:])
                    cmT_ps = ps_t.tile([C_OP, 128], f32, tag="tB")
                    nc.tensor.matmul(out=cmT_ps[:], lhsT=w1b_sb[:], rhs=cenT_sb[:],
                                     start=True, stop=True)

                    # ball query: T = valid * cumsum(valid) marks slot ranks
                    d_sb = sb.tile([128, PFX], f32, tag="d")
                    nc.sync.dma_start(d_sb[:], dist[r0:r1, :])
                    validf = sb.tile([128, PFX], f32, tag="valid")
                    nc.vector.tensor_scalar(validf[:], d_sb[:], R2, None,
                                            op0=mybir.AluOpType.is_lt)
                    rank = sb.tile([128, PFX], f32, tag="rank")
                    nc.vector.tensor_tensor_scan(rank[:], validf[:], zeros[:], 0.0,
                                                 op0=mybir.AluOpType.add,
                                                 op1=mybir.AluOpType.add)
                    nc.vector.tensor_copy(cnt_sb[:, t:t + 1], rank[:, PFX - 1:PFX])
                    tsl = sb.tile([128, PFX], f32, tag="tsl")
                    nc.gpsimd.tensor_mul(tsl[:], validf[:], rank[:])

                    # T^T chunks (n on partitions, centers on free)
                    tt = []
                    for xt in range(NXT):
                        tt_ps = ps_t.tile([128, 128], f32, tag="tA")
                        nc.tensor.transpose(
                            out=tt_ps[:], in_=tsl[:, xt * 128:(xt + 1) * 128],
                            identity=ident[:])
                        tt_sb = sb.tile([128, 128], f32, tag=f"tt{xt}")
                        nc.scalar.copy(tt_sb[:], tt_ps[:])
                        tt.append(tt_sb)

                    # slot-onehot matmuls: psum[jg][c, 4*128] = H rows per slot.
                    # Two independent max chains halve the serial PSUM-read
                    # dependency on DVE.
                    acc0 = sb.tile([C_OP, 4 * 128], f32, tag="acc0")
                    acc1 = sb.tile([C_OP, 4 * 128], f32, tag="acc1")
                    for jg in range(JG):
                        oh_ps = ps_oh.tile([C_OP, 4 * 128], f32, tag="oh_ps")
                        for xt in range(NXT):
                            oh = ohp.tile([128, 4 * 128], f32, tag="oh")
                            src = tt[xt]
                            b4 = bass.AP(src[:].tensor, src[:].offset,
                                         [list(src[:].ap[0]), [0, 4], [1, 128]])
                            nc.vector.tensor_tensor(
                                out=oh[:].rearrange("p (a b) -> p a b", a=4),
                                in0=b4,
                                in1=cj[:, jg * 512:(jg + 1) * 512].rearrange(
                                    "p (a b) -> p a b", a=4),
                                op=mybir.AluOpType.is_equal)
                            nc.tensor.matmul(out=oh_ps[:], lhsT=hc[xt][:],
                                             rhs=oh[:], start=(xt == 0),
                                             stop=(xt == NXT - 1))
                        acc = acc0 if jg % 2 == 0 else acc1
                        if jg < 2:
                            nc.scalar.copy(acc[:], oh_ps[:])
                        else:
                            nc.vector.tensor_tensor(out=acc[:], in0=acc[:],
                                                    in1=oh_ps[:],
                                                    op=mybir.AluOpType.max)

                    # merge chains, max over the 4 slots, subtract Cm'^T
                    nc.vector.tensor_tensor(out=acc0[:], in0=acc0[:], in1=acc1[:],
                                            op=mybir.AluOpType.max)
                    nc.vector.tensor_tensor(out=acc0[:, 0:256], in0=acc0[:, 0:256],
                                            in1=acc0[:, 256:512],
                                            op=mybir.AluOpType.max)
                    pT_sb = sb.tile([C_OP + 1, 128], f32, tag="pT_sb")
                    nc.vector.tensor_tensor(out=acc0[:, 0:128], in0=acc0[:, 0:128],
                                            in1=acc0[:, 128:256],
                                            op=mybir.AluOpType.max)
                    nc.vector.tensor_sub(pT_sb[0:C_OP, :], acc0[:, 0:128], cmT_ps[:])
                    nc.vector.memset(pT_sb[C_OP:C_OP + 1, :], 1.0)

                    o_ps = ps_o.tile([128, C_OUT], f32, tag="o")
                    nc.tensor.matmul(out=o_ps[:], lhsT=pT_sb[:], rhs=waggb_sb[:],
                                     start=True, stop=True)
                    o_sb = sb.tile([128, C_OUT], f32, tag="o_sb")
                    nc.scalar.activation(o_sb[:], o_ps[:],
                                         mybir.ActivationFunctionType.Relu)
                    nc.sync.dma_start(out[r0:r1, :], o_sb[:])

                nc.sync.dma_start(cnt[:], cnt_sb[:])

    nc.compile()
    return nc


def _get_program():
    global _PROG
    if _PROG is None:
        _PROG = _build_program()
    return _PROG


def _make_in_maps(positions, features, centers, distances, W_op, b_op, W_agg, b_agg):
    f = np.float32
    xpfx_by_b = [
        np.ascontiguousarray(
            np.concatenate([positions[b, :PFX], features[b, :PFX]], axis=-1), f)
        for b in range(B)
    ]
    w1b = np.ascontiguousarray(np.concatenate([W_op[:D], b_op[None]], 0), f)
    waggb = np.ascontiguousarray(np.concatenate([W_agg, b_agg[None]], 0), f)
    wop = np.ascontiguousarray(W_op, f)
    in_maps = []
    for c in range(NCORES):
        b, h = divmod(c, 2)
        m0 = h * MC
        in_maps.append({
            "dist": np.ascontiguousarray(distances[b, m0:m0 + MC, :PFX], f),
            "xpfx": xpfx_by_b[b],
            "cen": np.ascontiguousarray(centers[b, m0:m0 + MC], f),
            "wop": wop,
            "w1b": w1b,
            "waggb": waggb,
        })
    return in_maps


def _fallback_row(b, m, positions, features, centers, distances,
                  W_op, b_op, W_agg, b_agg):
    """Exact reference recompute of one output row (rare path)."""
    row = distances[b, m]
    idxs = np.nonzero(row < R2)[0][:K]
    f = np.zeros((K, C_OP), np.float32)
    if len(idxs):
        x = np.concatenate(
            [positions[b, idxs] - centers[b, m], features[b, idxs]], axis=-1)
        f[:len(idxs)] = x @ W_op + b_op
    pooled = f.max(0)
    return np.maximum(pooled @ W_agg + b_agg, 0).astype(np.float32)


def run(inputs, trace=False):
    """Run on the 8 NeuronCores; returns (full_output, BassKernelResults)."""
    from concourse.bass_utils import run_bass_kernel_spmd

    nc = _get_program()
    in_maps = _make_in_maps(**inputs)
    res = run_bass_kernel_spmd(nc, in_maps, core_ids=list(range(NCORES)),
                               trace=trace)

    out_full = np.zeros((B, M, C_OUT), np.float32)
    for c in range(NCORES):
        b, h = divmod(c, 2)
        m0 = h * MC
        out_full[b, m0:m0 + MC] = res.results[c]["out"]
        counts = res.results[c]["cnt"]  # [128, NT]; center t*128+p -> [p, t]
        deficient = np.nonzero(counts < K)
        for p, t in zip(*deficient):
            m = m0 + t * 128 + int(p)
            out_full[b, m] = _fallback_row(b, m, **inputs)
    return out_full, res


def kernel(**inputs):
    out, _ = run(inputs)
    return out

